# revision 7
# baseline (speedup 1.0000x reference)
"""BERT backbone (6 layers, sparse unit-diff attention mask) on 8 Trainium2 cores.

Sharding: data-parallel over batch (32 batches -> 4 per core). Each core runs
the full 6-layer stack on its 4 sequences; weights are replicated.

Device kernel layout strategy:
- token-major f32 masters for residual/LN (x), bounced through DRAM between
  layers; feature-major bf16 copies (xT) feed the GEMMs (PE transposes).
- GEMMs run in bf16 with f32 PSUM accumulation.
- attention per (batch, head-pair): scores q-major -> masked softmax (f32
  psum + ACT exp with fused row-sum) -> bf16 probs -> PE-transpose ->
  ctx = attnT.T @ V packed two heads per PSUM tile via tile_position.
"""

import os
import numpy as np
import ml_dtypes

B, S, H, NH, DH, II, L = 32, 512, 768, 12, 64, 3072, 6
NCORES = 8
BL = B // NCORES          # sequences per core
T = BL * S                # tokens per core
P = 128
KH = H // P               # 6 feature chunks
KI = II // P              # 24 intermediate chunks
QC = S // P               # 4 q-chunks per sequence
EPS = 1e-12
NEG = -262144.0           # additive mask value (pre-scale); exact in bf16
SCALE = 1.0 / np.sqrt(DH)

_CACHE = {}


def _build(n_layers=L, debug=False):
    import concourse.bass as bass
    import concourse.tile as tile
    from concourse import bacc, mybir
    from concourse.masks import make_identity

    f32 = mybir.dt.float32
    bf16 = mybir.dt.bfloat16
    Act = mybir.ActivationFunctionType

    nc = bacc.Bacc("TRN2", target_bir_lowering=False, debug=False,
                   num_devices=NCORES)

    # ---- DRAM I/O ----
    x0_d = nc.declare_dram_parameter("x0", [T, H], f32, isOutput=False).ap()
    mask_d = nc.declare_dram_parameter("mask", [S, S], bf16, isOutput=False).ap()
    out_d = nc.declare_dram_parameter("out", [T, H], f32, isOutput=True).ap()

    wd, bd = [], []
    for l in range(n_layers):
        wd.append({
            'wq': nc.declare_dram_parameter(f"wq{l}", [H, H], bf16, isOutput=False).ap(),
            'wk': nc.declare_dram_parameter(f"wk{l}", [H, H], bf16, isOutput=False).ap(),
            'wv': nc.declare_dram_parameter(f"wv{l}", [H, H], bf16, isOutput=False).ap(),
            'wo': nc.declare_dram_parameter(f"wo{l}", [H, H], bf16, isOutput=False).ap(),
            'wi': nc.declare_dram_parameter(f"wi{l}", [H, II], bf16, isOutput=False).ap(),
            'wf': nc.declare_dram_parameter(f"wf{l}", [II, H], bf16, isOutput=False).ap(),
        })
        bd.append({k: nc.declare_dram_parameter(f"{k}{l}", [sz], f32, isOutput=False).ap()
                   for k, sz in [('bq', H), ('bk', H), ('bv', H), ('bo', H),
                                 ('bi', II), ('bf', H),
                                 ('g1', H), ('b1', H), ('g2', H), ('b2', H)]})

    dbg = {}
    if debug:
        def dbgt(name, shape, dt_=bf16):
            dbg[name] = nc.declare_dram_parameter(name, shape, dt_, isOutput=True).ap()
        dbgt("d_xT", [KH, P, S])
        dbgt("d_qT", [KH, P, S]); dbgt("d_kT", [KH, P, S])
        dbgt("d_v", [QC, P, H])
        dbgt("d_attn", [QC, P, S]); dbgt("d_attnT", [QC, P, S])
        dbgt("d_ctxT", [KH, P, S])
        dbgt("d_x1", [QC, P, H], f32)
        dbgt("d_hT", [KI, P, S])

    with tile.TileContext(nc) as tc, __import__("contextlib").ExitStack() as ctx:
        const = ctx.enter_context(tc.tile_pool(name="const", bufs=1))
        biasp = ctx.enter_context(tc.tile_pool(name="biasp", bufs=1))
        wqkvo = ctx.enter_context(tc.tile_pool(name="wqkvo", bufs=1))
        wip = ctx.enter_context(tc.tile_pool(name="wip", bufs=4))
        wfp = ctx.enter_context(tc.tile_pool(name="wfp", bufs=1))
        xtp = ctx.enter_context(tc.tile_pool(name="xtp", bufs=1))
        qtp = ctx.enter_context(tc.tile_pool(name="qtp", bufs=1))
        ktp = ctx.enter_context(tc.tile_pool(name="ktp", bufs=1))
        vp = ctx.enter_context(tc.tile_pool(name="vp", bufs=1))
        ctxp = ctx.enter_context(tc.tile_pool(name="ctxp", bufs=1))
        x1tp = ctx.enter_context(tc.tile_pool(name="x1tp", bufs=1))
        htp = ctx.enter_context(tc.tile_pool(name="htp", bufs=1))
        atp = ctx.enter_context(tc.tile_pool(name="atp", bufs=8))
        aqp = ctx.enter_context(tc.tile_pool(name="aqp", bufs=2))
        xres = ctx.enter_context(tc.tile_pool(name="xres", bufs=8))
        castp = ctx.enter_context(tc.tile_pool(name="castp", bufs=2))
        work = ctx.enter_context(tc.tile_pool(name="work", bufs=16))
        dram = ctx.enter_context(tc.tile_pool(name="dram", bufs=2, space="DRAM"))

        ps_gemm = ctx.enter_context(tc.tile_pool(name="ps_gemm", bufs=2, space="PSUM"))
        ps_score = ctx.enter_context(tc.tile_pool(name="ps_score", bufs=2, space="PSUM"))
        ps_tr = ctx.enter_context(tc.tile_pool(name="ps_tr", bufs=2, space="PSUM"))
        ps_ctx = ctx.enter_context(tc.tile_pool(name="ps_ctx", bufs=2, space="PSUM"))

        # ---- constants ----
        ident32 = const.tile([P, P], f32)
        make_identity(nc, ident32)
        ident16 = const.tile([P, P], bf16)
        make_identity(nc, ident16)
        eps_t = const.tile([P, 1], f32)
        nc.vector.memset(eps_t, EPS)
        mask_sb = const.tile([P, QC, S], bf16)
        nc.sync.dma_start(out=mask_sb, in_=mask_d.rearrange("(c p) k -> p c k", p=P))

        def bcast(dram_ap, n, tag):
            t = biasp.tile([P, n], f32, tag=tag)
            src = bass.AP(tensor=dram_ap.tensor, offset=dram_ap.offset,
                          ap=[[0, P]] + list(dram_ap.ap))
            nc.sync.dma_start(out=t, in_=src)
            return t

        def percol(dram_ap, c, tag):
            # [c*P] -> [P, c] tile, column k holds dram[k*P:(k+1)*P]
            t = biasp.tile([P, c], f32, tag=tag)
            nc.sync.dma_start(out=t, in_=dram_ap.rearrange("(c p) -> p c", p=P))
            return t

        def transpose_to(dst_tiles, src_f32, trel, nch):
            # src f32 [128, nch*128] -> cast bf16 -> per chunk kc transpose into
            # dst_tiles[kc][:, trel*128:...]
            xc = castp.tile([P, nch * P], bf16, tag="xc", name="xc")
            nc.vector.tensor_copy(xc, src_f32)
            for kc in range(nch):
                tr16 = ps_tr.tile([P, P], bf16, tag="tr16", name="tr16")
                nc.tensor.transpose(out=tr16, in_=xc[:, kc * P:(kc + 1) * P],
                                    identity=ident16)
                nc.vector.tensor_copy(dst_tiles[kc][:, trel * P:(trel + 1) * P], tr16)

        def layer_norm(xt, mean_var_pool, g_t, b_t):
            stats = mean_var_pool.tile([P, 3, 6], f32, tag="stats")
            for sg in range(3):
                nc.vector.bn_stats(out=stats[:, sg, :], in_=xt[:, sg * 256:(sg + 1) * 256])
            mv = mean_var_pool.tile([P, 2], f32, tag="mv")
            nc.vector.bn_aggr(out=mv, in_=stats)
            rstd = mean_var_pool.tile([P, 1], f32, tag="rstd")
            nc.scalar.activation(out=rstd, in_=mv[:, 1:2], func=Act.Sqrt,
                                 bias=eps_t, scale=1.0)
            nc.vector.reciprocal(out=rstd, in_=rstd)
            nc.vector.tensor_scalar(out=xt, in0=xt, scalar1=mv[:, 0:1], scalar2=rstd,
                                    op0=mybir.AluOpType.subtract,
                                    op1=mybir.AluOpType.mult)
            nc.vector.tensor_tensor(out=xt, in0=xt, in1=g_t, op=mybir.AluOpType.mult)
            nc.vector.tensor_tensor(out=xt, in0=xt, in1=b_t, op=mybir.AluOpType.add)

        x_prev_ap = x0_d  # DRAM AP of the layer input master [T, H] f32

        for l in range(n_layers):
            w = wd[l]
            bb = bd[l]
            # ---- per-layer weight/bias loads (QKVO + Wf resident) ----
            wq_sb = [wqkvo.tile([P, H], bf16, tag=f"wq{kc}", name=f"wq{kc}") for kc in range(KH)]
            wk_sb = [wqkvo.tile([P, H], bf16, tag=f"wk{kc}", name=f"wk{kc}") for kc in range(KH)]
            wv_sb = [wqkvo.tile([P, H], bf16, tag=f"wv{kc}", name=f"wv{kc}") for kc in range(KH)]
            wo_sb = [wqkvo.tile([P, H], bf16, tag=f"wo{kc}", name=f"wo{kc}") for kc in range(KH)]
            for kc in range(KH):
                nc.sync.dma_start(out=wq_sb[kc], in_=w['wq'][kc * P:(kc + 1) * P, :])
                nc.sync.dma_start(out=wk_sb[kc], in_=w['wk'][kc * P:(kc + 1) * P, :])
                nc.sync.dma_start(out=wv_sb[kc], in_=w['wv'][kc * P:(kc + 1) * P, :])
                nc.sync.dma_start(out=wo_sb[kc], in_=w['wo'][kc * P:(kc + 1) * P, :])
            wf_sb = [wfp.tile([P, H], bf16, tag=f"wf{kc}", name=f"wf{kc}") for kc in range(KI)]
            for kc in range(KI):
                nc.sync.dma_start(out=wf_sb[kc], in_=w['wf'][kc * P:(kc + 1) * P, :])

            bqk_sb = percol(bd[l]['bq'], KH, "bq")
            bk_sb = percol(bd[l]['bk'], KH, "bk")
            bv_sb = percol(bd[l]['bv'], KH, "bv")
            bi_sb = percol(bd[l]['bi'], KI, "bi")
            bo_b = bcast(bb['bo'], H, "bo")
            bf_b = bcast(bb['bf'], H, "bf")
            g1_b = bcast(bb['g1'], H, "g1")
            b1_b = bcast(bb['b1'], H, "b1")
            g2_b = bcast(bb['g2'], H, "g2")
            b2_b = bcast(bb['b2'], H, "b2")

            last = (l == n_layers - 1)
            x_next = None if last else dram.tile([T, H], f32, tag="xm")

            for b in range(BL):
                t0 = b * S
                # ---- T0: load x tiles, build xT (bf16 feature-major) ----
                x_f32 = []
                for trel in range(QC):
                    xt_ = xres.tile([P, H], f32, tag="xr")
                    nc.sync.dma_start(out=xt_, in_=x_prev_ap[t0 + trel * P:t0 + (trel + 1) * P, :])
                    x_f32.append(xt_)
                xT = [xtp.tile([P, S], bf16, tag=f"xT{kc}", name=f"xT{kc}") for kc in range(KH)]
                for trel in range(QC):
                    transpose_to(xT, x_f32[trel], trel, KH)

                # ---- QKV ----
                qT = [qtp.tile([P, S], bf16, tag=f"qT{kc}", name=f"qT{kc}") for kc in range(KH)]
                kT = [ktp.tile([P, S], bf16, tag=f"kT{kc}", name=f"kT{kc}") for kc in range(KH)]
                for dst, wsb, bsb in ((qT, wq_sb, bqk_sb), (kT, wk_sb, bk_sb)):
                    for jc in range(KH):
                        ps = ps_gemm.tile([P, S], f32, tag="gemm")
                        for kc in range(KH):
                            nc.tensor.matmul(ps, lhsT=wsb[kc][:, jc * P:(jc + 1) * P],
                                             rhs=xT[kc], start=(kc == 0), stop=(kc == KH - 1))
                        nc.scalar.activation(out=dst[jc], in_=ps, func=Act.Identity,
                                             bias=bsb[:, jc:jc + 1], scale=1.0)
                v_sb = [vp.tile([P, H], bf16, tag=f"v{trel}", name=f"v{trel}") for trel in range(QC)]
                for trel in range(QC):
                    for jg, (j0, jw) in enumerate(((0, 512), (512, 256))):
                        ps = ps_gemm.tile([P, S], f32, tag="gemm")
                        for kc in range(KH):
                            nc.tensor.matmul(ps[:, :jw],
                                             lhsT=xT[kc][:, trel * P:(trel + 1) * P],
                                             rhs=wv_sb[kc][:, j0:j0 + jw],
                                             start=(kc == 0), stop=(kc == KH - 1))
                        nc.vector.tensor_copy(v_sb[trel][:, j0:j0 + jw], ps[:, :jw])

                # ---- attention, head pairs ----
                ctxT = [ctxp.tile([P, S], bf16, tag=f"c{kc}", name=f"c{kc}") for kc in range(KH)]
                for g in range(NH // 2):
                    at_pair = []
                    for hh in range(2):
                        h = 2 * g + hh
                        kc_h, ro = h // 2, (h % 2) * DH
                        q_h = qT[kc_h][ro:ro + DH, :]
                        k_h = kT[kc_h][ro:ro + DH, :]
                        at_h = [atp.tile([P, S], bf16, tag="at", name="at") for _ in range(QC)]
                        for qc in range(QC):
                            ps = ps_score.tile([P, S], f32, tag="sc")
                            nc.tensor.matmul(ps, lhsT=q_h[:, qc * P:(qc + 1) * P],
                                             rhs=k_h, start=True, stop=True)
                            nc.vector.tensor_tensor(out=ps, in0=ps,
                                                    in1=mask_sb[:, qc, :],
                                                    op=mybir.AluOpType.add)
                            negmax = work.tile([P, 1], f32, tag="negmax")
                            nc.vector.reduce_max(out=negmax, in_=ps,
                                                 axis=mybir.AxisListType.X, negate=True)
                            nm2 = work.tile([P, 1], f32, tag="nm2")
                            nc.scalar.mul(out=nm2, in_=negmax, mul=SCALE)
                            aq = aqp.tile([P, S], bf16, tag="aq")
                            lsum = work.tile([P, 1], f32, tag="lsum")
                            nc.scalar.activation(out=aq, in_=ps, func=Act.Exp,
                                                 bias=nm2, scale=SCALE, accum_out=lsum)
                            rl = work.tile([P, 1], f32, tag="rl")
                            nc.vector.reciprocal(out=rl, in_=lsum)
                            nc.vector.tensor_scalar_mul(out=aq, in0=aq, scalar1=rl)
                            if debug and l == 0 and b == 0 and h == 0:
                                nc.sync.dma_start(out=dbg["d_attn"][qc], in_=aq)
                            for kc in range(QC):
                                tr16 = ps_tr.tile([P, P], bf16, tag="tr16")
                                nc.tensor.transpose(out=tr16, in_=aq[:, kc * P:(kc + 1) * P],
                                                    identity=ident16)
                                nc.vector.tensor_copy(at_h[kc][:, qc * P:(qc + 1) * P], tr16)
                        at_pair.append(at_h)
                    psc = ps_ctx.tile([P, S], f32, tag="ctx")
                    for hh in range(2):
                        h = 2 * g + hh
                        ro = hh * DH
                        for kc in range(QC):
                            nc.tensor.matmul(psc[ro:ro + DH, :],
                                             lhsT=v_sb[kc][:, h * DH:(h + 1) * DH],
                                             rhs=at_pair[hh][kc],
                                             start=(kc == 0), stop=(kc == QC - 1),
                                             tile_position=(0, ro))
                    nc.scalar.activation(out=ctxT[g], in_=psc, func=Act.Identity,
                                         bias=bv_sb[:, g:g + 1], scale=1.0)
                    if debug and l == 0 and b == 0 and g == 0:
                        for kc in range(QC):
                            nc.sync.dma_start(out=dbg["d_attnT"][kc], in_=at_pair[0][kc])

                # ---- Wo + residual + LN1 ----
                x1 = []
                for trel in range(QC):
                    x1t = xres.tile([P, H], f32, tag="xr")
                    for jg, (j0, jw) in enumerate(((0, 512), (512, 256))):
                        ps = ps_gemm.tile([P, S], f32, tag="gemm")
                        for kc in range(KH):
                            nc.tensor.matmul(ps[:, :jw],
                                             lhsT=ctxT[kc][:, trel * P:(trel + 1) * P],
                                             rhs=wo_sb[kc][:, j0:j0 + jw],
                                             start=(kc == 0), stop=(kc == KH - 1))
                        nc.vector.tensor_tensor(out=x1t[:, j0:j0 + jw], in0=ps[:, :jw],
                                                in1=bo_b[:, j0:j0 + jw],
                                                op=mybir.AluOpType.add)
                        nc.vector.tensor_tensor(out=x1t[:, j0:j0 + jw],
                                                in0=x1t[:, j0:j0 + jw],
                                                in1=x_f32[trel][:, j0:j0 + jw],
                                                op=mybir.AluOpType.add)
                    layer_norm(x1t, work, g1_b, b1_b)
                    x1.append(x1t)
                x1T = [x1tp.tile([P, S], bf16, tag=f"x1T{kc}", name=f"x1T{kc}") for kc in range(KH)]
                for trel in range(QC):
                    transpose_to(x1T, x1[trel], trel, KH)

                # ---- FFN ----
                hT = [htp.tile([P, S], bf16, tag=f"h{kc}", name=f"h{kc}") for kc in range(KI)]
                for jc in range(KI):
                    wi_t = wip.tile([P, KH, P], bf16, tag="wi")
                    nc.sync.dma_start(
                        out=wi_t,
                        in_=w['wi'].rearrange("(c p) j -> p c j", p=P)[:, :, jc * P:(jc + 1) * P])
                    ps = ps_gemm.tile([P, S], f32, tag="gemm")
                    for kc in range(KH):
                        nc.tensor.matmul(ps, lhsT=wi_t[:, kc, :], rhs=x1T[kc],
                                         start=(kc == 0), stop=(kc == KH - 1))
                    nc.scalar.activation(out=hT[jc], in_=ps, func=Act.Gelu,
                                         bias=bi_sb[:, jc:jc + 1], scale=1.0)
                for trel in range(QC):
                    x2t = xres.tile([P, H], f32, tag="xr")
                    for jg, (j0, jw) in enumerate(((0, 512), (512, 256))):
                        ps = ps_gemm.tile([P, S], f32, tag="gemm")
                        for kc in range(KI):
                            nc.tensor.matmul(ps[:, :jw],
                                             lhsT=hT[kc][:, trel * P:(trel + 1) * P],
                                             rhs=wf_sb[kc][:, j0:j0 + jw],
                                             start=(kc == 0), stop=(kc == KI - 1))
                        nc.vector.tensor_tensor(out=x2t[:, j0:j0 + jw], in0=ps[:, :jw],
                                                in1=bf_b[:, j0:j0 + jw],
                                                op=mybir.AluOpType.add)
                        nc.vector.tensor_tensor(out=x2t[:, j0:j0 + jw],
                                                in0=x2t[:, j0:j0 + jw],
                                                in1=x1[trel][:, j0:j0 + jw],
                                                op=mybir.AluOpType.add)
                    layer_norm(x2t, work, g2_b, b2_b)
                    dst = out_d if last else x_next
                    nc.sync.dma_start(out=dst[t0 + trel * P:t0 + (trel + 1) * P, :],
                                      in_=x2t)

                if debug and l == 0 and b == 0:
                    for kc in range(KH):
                        nc.sync.dma_start(out=dbg["d_xT"][kc], in_=xT[kc])
                        nc.sync.dma_start(out=dbg["d_qT"][kc], in_=qT[kc])
                        nc.sync.dma_start(out=dbg["d_kT"][kc], in_=kT[kc])
                        nc.sync.dma_start(out=dbg["d_ctxT"][kc], in_=ctxT[kc])
                    for trel in range(QC):
                        nc.sync.dma_start(out=dbg["d_v"][trel], in_=v_sb[trel])
                        nc.sync.dma_start(out=dbg["d_x1"][trel], in_=x1[trel])
                    for kc in range(KI):
                        nc.sync.dma_start(out=dbg["d_hT"][kc], in_=hT[kc])

            x_prev_ap = None if last else x_next

    nc.compile()
    return nc


def _host_embed(input_ids, params):
    ids = np.asarray(input_ids)
    we = np.asarray(params['word_emb'], np.float32)
    pe = np.asarray(params['pos_emb'], np.float32)
    te = np.asarray(params['type_emb'], np.float32)
    x = we[ids] + pe[None, :S, :] + te[0][None, None, :]
    g = np.asarray(params['emb_ln_g'], np.float32)
    bta = np.asarray(params['emb_ln_b'], np.float32)
    mu = x.mean(-1, keepdims=True)
    var = ((x - mu) ** 2).mean(-1, keepdims=True)
    x = (x - mu) / np.sqrt(var + EPS) * g + bta
    return x.astype(np.float32)


def _host_mask(unit_size, n):
    unit_size = int(unit_size)
    n = int(n)
    idx = np.arange(S)
    x0 = idx >= n
    unit = np.where(x0, (idx - n) // unit_size, idx // unit_size)
    uq, ukv = unit[:, None], unit[None, :]
    xq, xkv = x0[:, None], x0[None, :]
    m = ((uq == ukv) & (xq == xkv)) | ((uq > ukv) & xkv & (~xq)) | \
        ((uq >= ukv) & xkv & xq)
    return np.where(m, np.float32(0.0), np.float32(NEG)).astype(ml_dtypes.bfloat16)


def _in_maps(input_ids, params, unit_size, n, n_layers=L):
    x0 = _host_embed(input_ids, params)          # [B, S, H] f32
    mask = _host_mask(unit_size, n)
    lay = params['layers']
    shared = {"mask": mask}
    for l in range(n_layers):
        for k, src in [('wq', 'Wq'), ('wk', 'Wk'), ('wv', 'Wv'), ('wo', 'Wo'),
                       ('wi', 'Wi'), ('wf', 'Wf')]:
            shared[f"{k}{l}"] = np.asarray(lay[src][l]).astype(ml_dtypes.bfloat16)
        for k, src in [('bq', 'bq'), ('bk', 'bk'), ('bv', 'bv'), ('bo', 'bo'),
                       ('bi', 'bi'), ('bf', 'bf'), ('g1', 'ln1_g'), ('b1', 'ln1_b'),
                       ('g2', 'ln2_g'), ('b2', 'ln2_b')]:
            shared[f"{k}{l}"] = np.asarray(lay[src][l], np.float32)
    maps = []
    for c in range(NCORES):
        m = dict(shared)
        m["x0"] = x0[c * BL:(c + 1) * BL].reshape(T, H)
        maps.append(m)
    return maps


def kernel(input_ids, params, unit_size, n):
    from concourse.bass_utils import run_bass_kernel_spmd
    key = ("nc", L)
    if key not in _CACHE:
        _CACHE[key] = _build(L, debug=False)
    nc = _CACHE[key]
    maps = _in_maps(input_ids, params, unit_size, n, L)
    res = run_bass_kernel_spmd(nc, maps, core_ids=list(range(NCORES)))
    out = np.stack([r["out"].reshape(BL, S, H) for r in res.results], 0)
    return out.reshape(B, S, H).astype(np.float32)
